# revision 6
# baseline (speedup 1.0000x reference)
"""Grid pooling (segment mean over rectangular grid cells) on 8 trn2 cores.

Math: row/col masks induce contiguous run-segments along H and W, so every
grid cell is a rectangle and the whole op factorizes per channel as

    out_c = A_h @ ( A_h'^T @ X_c @ A_w' ) @ A_w^T

with segment-assignment matrices built on host from the tiny masks. The
mean scale 1/(n_r*n_q) is separable, so 1/n_r is baked into the stage-1
weights A_h' and 1/n_q into the stage-2 weights A_w' (both bf16); stages
3/4 use exact one-hot gathers. Channels (64) are sharded 8-way across
cores; each core runs 8 independent 768x768 channel planes:

  1. R^T = X_c^T @ A_h'   per (w-tile m, h-chunk k): only the ~33 row
     segments LOCAL to chunk k are streamed (out columns = per-chunk
     "slots" at disjoint offsets; no PSUM accumulation). A row segment
     straddling two chunks gets two partial-sum slots; since each slot
     already carries the full 1/(n_r*n_q) scale downstream, the partials
     are merged for free by 1s in BOTH slot rows of the stage-3/4 gather
     matrices (mean = sum_A/n + sum_B/n).
  2. S^T = A_w'^T @ R^T   per q-block (contract W in 6 chunks).
  3. U   = S^T-gather cols (adapted r-slot blocks, single-k matmuls)
  4. OUT = A_h-gather rows (per h-chunk, lhsT rows = that tile's slots)

Data is bf16 on-chip (fp32 PSUM accumulation); PSUM->SBUF casts are
round-robined across Vector, Scalar(Act) and GpSimd(Pool) engines so no
single engine bottlenecks. The program is cached per blocking structure.
"""

import numpy as np
import ml_dtypes

from concourse import bacc, tile
import concourse.mybir as mybir
from concourse.bass_utils import run_bass_kernel_spmd

H = 768
W = 768
C = 64
NCORES = 8
CL = C // NCORES          # channels per core
HK = H // 128             # 6 H-chunks
WK = W // 128             # 6 W-chunks
NB = 384                  # free-dim tile for broadcast stages (768 = 2*384)

DT = mybir.dt.bfloat16
F32 = mybir.dt.float32
NPDT = ml_dtypes.bfloat16

_cached = {}


def _segment_ids(mask: np.ndarray) -> np.ndarray:
    """mask [L] binary -> segment ids via rising edges (pixel 0 -> seg 0)."""
    m = mask.astype(np.int64)
    prev = np.concatenate([[0], m[:-1]])
    rising = (m == 1) & (prev == 0)
    rising[0] = False
    return np.cumsum(rising.astype(np.int64)).astype(np.int32)


def _adapt_blocks(ids: np.ndarray, tile_len: int, nseg: int):
    """Partition the segment axis into blocks of <=128 ids such that the id
    range of every output tile of `tile_len` positions lies in one block.
    Falls back to fixed disjoint 128-blocks (multi-k) if a range is too wide.
    """
    L = len(ids)
    ntiles = L // tile_len
    ranges = [(int(ids[t * tile_len]), int(ids[(t + 1) * tile_len - 1]))
              for t in range(ntiles)]
    if all(hi - lo + 1 <= 128 for lo, hi in ranges):
        blocks, assign = _greedy_blocks(ranges)
        return blocks, [[a] for a in assign]
    blocks = [(off, min(128, nseg - off)) for off in range(0, nseg, 128)]
    assign = []
    for lo, hi in ranges:
        assign.append([b for b, (s, w) in enumerate(blocks)
                       if not (hi < s or lo > s + w - 1)])
    return blocks, assign


def _greedy_blocks(ranges):
    """Greedy cover of per-tile index ranges (each <=128 wide) by blocks of
    <=128 consecutive indices; blocks may overlap. Returns (blocks, assign)."""
    blocks, assign = [], []
    cur_start, cur_end = 0, -1
    for lo, hi in ranges:
        if cur_end >= 0 and hi - cur_start + 1 <= 128:
            cur_end = max(cur_end, hi)
        else:
            if cur_end >= 0:
                blocks.append((cur_start, cur_end - cur_start + 1))
            cur_start, cur_end = lo, hi
        assign.append(len(blocks))
    blocks.append((cur_start, cur_end - cur_start + 1))
    return blocks, assign


def _row_slots(row_ids: np.ndarray):
    """Per-h-chunk local slot layout for stage 1.

    Returns (offs, widths, NRS, ranges) where chunk k's local segments
    [lo_k, hi_k] map to slots [offs[k], offs[k]+widths[k]); a segment
    spanning chunks holds one (partial-sum) slot per chunk. ranges[m] is
    the slot range tile m's output pixels touch, including the cross-chunk
    partial slots of its boundary segments.
    """
    los = [int(row_ids[k * 128]) for k in range(HK)]
    his = [int(row_ids[(k + 1) * 128 - 1]) for k in range(HK)]
    widths = [his[k] - los[k] + 1 for k in range(HK)]
    offs = list(np.concatenate([[0], np.cumsum(widths)[:-1]]).astype(int))
    nrs = int(sum(widths))
    ranges = []
    for m in range(HK):
        if m > 0 and los[m] <= his[m - 1]:
            smin = offs[m - 1] + (los[m] - los[m - 1])
        else:
            smin = offs[m]
        if m < HK - 1 and his[m] >= los[m + 1]:
            smax = offs[m + 1] + (his[m] - los[m + 1])
        else:
            smax = offs[m] + widths[m] - 1
        ranges.append((smin, smax))
    return offs, widths, nrs, los, his, ranges


def _build_program(key):
    (NRS, WMAXR, offs, widths, NCP, rblocks, rassign, qblocks, qassign) = key
    RBn = len(rblocks)
    QBn = len(qblocks)

    nc = bacc.Bacc("TRN2", target_bir_lowering=False, debug=False,
                   num_devices=NCORES)

    x_d = nc.dram_tensor("x", [CL, 128, HK, W], DT, kind="ExternalInput")
    ahn_d = nc.dram_tensor("ahn", [128, HK, WMAXR], DT, kind="ExternalInput")
    awn_d = nc.dram_tensor("awn", [128, WK, NCP], DT, kind="ExternalInput")
    awtb_d = nc.dram_tensor("awtb", [128, QBn, W], DT, kind="ExternalInput")
    ahtb_d = nc.dram_tensor("ahtb", [128, RBn, H], DT, kind="ExternalInput")
    o_d = nc.dram_tensor("o", [CL, 128, HK, W], DT, kind="ExternalOutput")

    with tile.TileContext(nc) as tc:
        with (
            tc.tile_pool(name="const", bufs=1) as constp,
            tc.tile_pool(name="xp", bufs=3) as xp,
            tc.tile_pool(name="rp", bufs=2) as rp,
            tc.tile_pool(name="sp", bufs=2) as sp,
            tc.tile_pool(name="up", bufs=2) as up,
            tc.tile_pool(name="op", bufs=2) as op_,
            # 8 PSUM banks total: psr 2 + pss 1 + psb 5 (stages 3+4 share)
            tc.tile_pool(name="psr", bufs=2, space="PSUM") as psr,
            tc.tile_pool(name="pss", bufs=1, space="PSUM") as pss,
            tc.tile_pool(name="psb", bufs=5, space="PSUM") as psb,
        ):
            # first channel's input before the big broadcast constants so the
            # PE can start as early as possible (ahn is all stage 1 needs);
            # halves land on different DMA queues and overlap
            xc0 = xp.tile([128, HK, W], DT)
            nc.sync.dma_start(xc0[:, 0:HK // 2, :], x_d[0][:, 0:HK // 2, :])
            ahn = constp.tile([128, HK, WMAXR], DT)
            nc.sync.dma_start(ahn[:], ahn_d[:])
            nc.sync.dma_start(xc0[:, HK // 2:HK, :], x_d[0][:, HK // 2:HK, :])
            awn = constp.tile([128, WK, NCP], DT)
            nc.sync.dma_start(awn[:], awn_d[:])
            awtb = constp.tile([128, QBn, W], DT)
            nc.sync.dma_start(awtb[:], awtb_d[:])
            ahtb = constp.tile([128, RBn, H], DT)
            nc.sync.dma_start(ahtb[:], ahtb_d[:])

            # PSUM->SBUF casts alternate between Vector and Act so neither
            # bottlenecks (GpSimd cannot read PSUM on TRN2)
            def copier(eng):
                if eng % 2 == 0:
                    return nc.vector.tensor_copy
                return nc.scalar.copy

            for c in range(CL):
                if c == 0:
                    xc = xc0
                else:
                    xc = xp.tile([128, HK, W], DT)
                    nc.sync.dma_start(xc[:], x_d[c])

                # stage 1: per w-tile m, 6 single-shot matmuls write the
                # chunk-local slot ranges of one PSUM bank (no accumulation)
                rc = rp.tile([128, WK, NRS], DT)
                for m in range(WK):
                    pr = psr.tile([128, NRS], F32)
                    for k in range(HK):
                        nc.tensor.matmul(
                            pr[:, offs[k]:offs[k] + widths[k]],
                            xc[:, k, 128 * m:128 * m + 128],
                            ahn[:, k, 0:widths[k]],
                            start=True, stop=True,
                        )
                    copier(m)(rc[:, m, :], pr[:])

                # stage 2: S^T[q, slot] per q-block (contract W in 6 chunks);
                # 1/n_q lives in awn so the copy-out needs no scaling
                sc = sp.tile([128, QBn, NRS], DT)
                for b, (qo, qs) in enumerate(qblocks):
                    ps = pss.tile([128, NRS], F32)
                    for k in range(WK):
                        nc.tensor.matmul(
                            ps[0:qs, :],
                            awn[:, k, qo:qo + qs],
                            rc[:, k, :],
                            start=(k == 0), stop=(k == WK - 1),
                        )
                    copier(b % 2)(sc[0:qs, b, :], ps[0:qs, :])

                # stage 3: U[slot, j] = S^T[col_ids(j), slot] per r-slot-block
                uc = up.tile([128, RBn, W], DT)
                for ri, (ro, rs) in enumerate(rblocks):
                    for n in range(W // NB):
                        pu = psb.tile([128, NB], F32, tag="pb", name="pu")
                        ks = qassign[n]
                        for j, b in enumerate(ks):
                            qo, qs = qblocks[b]
                            nc.tensor.matmul(
                                pu[0:rs, :],
                                sc[0:qs, b, ro:ro + rs],
                                awtb[0:qs, b, NB * n:NB * n + NB],
                                start=(j == 0), stop=(j == len(ks) - 1),
                            )
                        copier(2 * ri + n)(
                            uc[0:rs, ri, NB * n:NB * n + NB], pu[0:rs, :])

                # stage 4: OUT[i, j] = sum over rid[i]'s slots of U[slot, j];
                # output DMA'd in 2-h-chunk pieces
                ocC = op_.tile([128, HK, W], DT)
                for m in range(HK):
                    b = rassign[m]
                    ro, rs = rblocks[b]
                    for n in range(W // NB):
                        po = psb.tile([128, NB], F32, tag="pb", name="po")
                        nc.tensor.matmul(
                            po[:],
                            ahtb[0:rs, b, 128 * m:128 * m + 128],
                            uc[0:rs, b, NB * n:NB * n + NB],
                            start=True, stop=True,
                        )
                        copier(2 * m + n)(ocC[:, m, NB * n:NB * n + NB],
                                                po[:])
                    if m % 2 == 1 and m < HK - 1:
                        nc.sync.dma_start(o_d[c][:, m - 1:m + 1, :],
                                          ocC[:, m - 1:m + 1, :])
                nc.sync.dma_start(o_d[c][:, HK - 2:HK, :],
                                  ocC[:, HK - 2:HK, :])

    nc.compile()
    return nc


def _get_program(key):
    if key not in _cached:
        _cached[key] = _build_program(key)
    return _cached[key]


def _prepare(input, h_mask, v_mask):
    x = np.asarray(input, dtype=np.float32)
    hm = np.asarray(h_mask, dtype=np.int32)
    vm = np.asarray(v_mask, dtype=np.int32)
    assert x.shape == (1, H, W, C), x.shape

    row_ids = _segment_ids(hm[0])
    col_ids = _segment_ids(vm[0])
    nr = int(row_ids[-1]) + 1
    ncs = int(col_ids[-1]) + 1
    NCP = ((ncs + 63) // 64) * 64

    n_r = np.bincount(row_ids, minlength=nr).astype(np.float64)
    n_q = np.bincount(col_ids, minlength=NCP).astype(np.float64)

    # ---- row side: per-chunk slot layout (stage 1 locality) ----
    offs, widths, NRS, los, his, ranges = _row_slots(row_ids)
    assert NRS * 4 <= 2048, NRS   # stage-1/2 PSUM tile must fit one bank
    assert all(hi - lo + 1 <= 128 for lo, hi in ranges)
    rblocks, rassign = _greedy_blocks(ranges)
    WMAXR = max(widths)

    # ---- col side: global blocks as before ----
    qblocks, qassign = _adapt_blocks(col_ids, NB, ncs)

    key = (NRS, WMAXR, tuple(offs), tuple(widths), NCP,
           tuple(rblocks), tuple(rassign),
           tuple(qblocks), tuple(tuple(a) for a in qassign))

    # stage-1 weights: chunk-local one-hot scaled by 1/n_r
    ahn_dev = np.zeros((128, HK, WMAXR), np.float32)
    for k in range(HK):
        for p in range(128):
            r = int(row_ids[128 * k + p])
            ahn_dev[p, k, r - los[k]] = 1.0 / n_r[r]
    # stage-2 weights: one-hot scaled by 1/n_q
    awn_dev = np.zeros((128, WK, NCP), np.float32)
    for k in range(WK):
        for p in range(128):
            q = int(col_ids[128 * k + p])
            awn_dev[p, k, q] = 1.0 / n_q[q]
    # stage-3 gather: exact one-hot over global col segments (q-block rows)
    QBn = len(qblocks)
    awtb_dev = np.zeros((128, QBn, W), np.float32)
    for b, (qo, qs) in enumerate(qblocks):
        for j in range(W):
            q = int(col_ids[j])
            if qo <= q < qo + qs:
                awtb_dev[q - qo, b, j] = 1.0
    # stage-4 gather: 1s at ALL slots of pixel i's row segment (this is what
    # merges cross-chunk partial sums), rows local to the tile's slot block
    RBn = len(rblocks)
    ahtb_dev = np.zeros((128, RBn, H), np.float32)
    for i in range(H):
        r = int(row_ids[i])
        b = rassign[i // 128]
        ro, rs = rblocks[b]
        for k in range(HK):
            if los[k] <= r <= his[k]:
                s = offs[k] + r - los[k]
                assert ro <= s < ro + rs, (i, r, k, s, rblocks[b])
                ahtb_dev[s - ro, b, i] = 1.0

    ahn_dev = ahn_dev.astype(NPDT)
    awn_dev = awn_dev.astype(NPDT)
    awtb_dev = awtb_dev.astype(NPDT)
    ahtb_dev = ahtb_dev.astype(NPDT)

    # per-core planar input: [CL, 128(p), HK(h0), W] with h = 128*h0 + p
    x64 = x[0].transpose(2, 0, 1)  # [C, H, W]
    in_maps = []
    for core in range(NCORES):
        xc = x64[CL * core:CL * (core + 1)]  # [CL, H, W]
        xdev = np.ascontiguousarray(
            xc.reshape(CL, HK, 128, W).transpose(0, 2, 1, 3)).astype(NPDT)
        in_maps.append({
            "x": xdev,
            "ahn": ahn_dev,
            "awn": awn_dev,
            "awtb": awtb_dev,
            "ahtb": ahtb_dev,
        })
    return in_maps, key


def _assemble(results):
    out = np.empty((1, H, W, C), np.float32)
    for core in range(NCORES):
        o = np.asarray(results[core]["o"]).astype(np.float32)  # [CL,128,HK,W]
        oc = o.transpose(0, 2, 1, 3).reshape(CL, H, W)         # h = 128*m + p
        out[0, :, :, CL * core:CL * (core + 1)] = oc.transpose(1, 2, 0)
    return out


def run(inputs: dict, trace: bool = False, **kwargs):
    """Full pipeline; returns (output, BassKernelResults)."""
    in_maps, key = _prepare(**inputs)
    nc = _get_program(key)
    res = run_bass_kernel_spmd(nc, in_maps, list(range(NCORES)),
                               trace=trace, **kwargs)
    return _assemble(res.results), res


def kernel(**inputs) -> np.ndarray:
    out, _ = run(inputs, trace=False)
    return out


# revision 7
# speedup vs baseline: 1.0357x; 1.0357x over previous
"""Grid pooling (segment mean over rectangular grid cells) on 8 trn2 cores.

Math: row/col masks induce contiguous run-segments along H and W, so every
grid cell is a rectangle and the whole op factorizes per channel as

    out_c = A_h @ ( A_h'^T @ X_c @ A_w' ) @ A_w^T

with segment-assignment matrices built on host from the tiny masks. The
mean scale 1/(n_r*n_q) is separable: 1/n_r is baked into the stage-1
weights A_h' and 1/n_q into the stage-2 weights A_w' (bf16), so every
PSUM->SBUF move is a plain cast. Channels (64) are sharded 8-way across
cores; each core runs 8 independent 768x768 channel planes through 4
matmul stages:

  1. R^T  = X_c^T  @ A_h'   (row-segment sums;   lhsT = X chunks)
  2. S^T  = A_w'^T @ R^T    (col-segment means;  lhsT = A_w' chunks)
  3. U    = S^T-gather cols (adapted r-blocks, single-k matmuls)
  4. OUT  = A_h-gather rows (per h-chunk)

Stage-1/2 matmuls stream ~192 columns, which exactly hides the ~75ns
LDWEIGHTS of each 128-row weight load and keeps the PE p-state at full
clock. The channel loop is software-pipelined as

    s1(c) -> s3(c-1) -> s4(c-1) -> s2(c)

so the PE never waits for a PSUM->SBUF cast to finish (each stage's
consumer runs ~3us after its producer). Casts alternate between the
Vector and Act engines (GpSimd cannot read PSUM on TRN2); stages 3+4
share one 5-deep PSUM pool so casts never backpressure the PE.
"""

import numpy as np
import ml_dtypes

from concourse import bacc, tile
import concourse.mybir as mybir
from concourse.bass_utils import run_bass_kernel_spmd

H = 768
W = 768
C = 64
NCORES = 8
CL = C // NCORES          # channels per core
HK = H // 128             # 6 H-chunks (contraction / output chunks)
WK = W // 128             # 6 W-chunks
NB = 384                  # free-dim tile for broadcast stages (768 = 2*384)

DT = mybir.dt.bfloat16
F32 = mybir.dt.float32
NPDT = ml_dtypes.bfloat16

_cached = {}


def _segment_ids(mask: np.ndarray) -> np.ndarray:
    """mask [L] binary -> segment ids via rising edges (pixel 0 -> seg 0)."""
    m = mask.astype(np.int64)
    prev = np.concatenate([[0], m[:-1]])
    rising = (m == 1) & (prev == 0)
    rising[0] = False
    return np.cumsum(rising.astype(np.int64)).astype(np.int32)


def _adapt_blocks(ids: np.ndarray, tile_len: int, nseg: int):
    """Partition the segment axis into blocks of <=128 ids such that the id
    range of every output tile of `tile_len` positions lies in one block.

    Returns (blocks, assign): blocks = [(start, width), ...] (may overlap by
    a shared boundary segment), assign[t] = [(block_idx), ...] the block(s)
    tile t accumulates over. Single-element lists on the fast path; falls
    back to fixed disjoint 128-blocks (multi-k) if a range is too wide.
    """
    L = len(ids)
    ntiles = L // tile_len
    ranges = [(int(ids[t * tile_len]), int(ids[(t + 1) * tile_len - 1]))
              for t in range(ntiles)]
    if all(hi - lo + 1 <= 128 for lo, hi in ranges):
        blocks, assign = [], []
        cur_start, cur_end = 0, -1
        for lo, hi in ranges:
            if hi - cur_start + 1 <= 128 and cur_end >= 0:
                cur_end = max(cur_end, hi)
            else:
                if cur_end >= 0:
                    blocks.append((cur_start, cur_end - cur_start + 1))
                cur_start, cur_end = lo, hi
            assign.append(len(blocks))
        blocks.append((cur_start, cur_end - cur_start + 1))
        return blocks, [[a] for a in assign]
    blocks = [(off, min(128, nseg - off)) for off in range(0, nseg, 128)]
    assign = []
    for lo, hi in ranges:
        assign.append([b for b, (s, w) in enumerate(blocks)
                       if not (hi < s or lo > s + w - 1)])
    return blocks, assign


def _build_program(key):
    (NRP, NCP, rblocks, rassign, qblocks, qassign) = key
    RBn = len(rblocks)
    QBn = len(qblocks)

    nc = bacc.Bacc("TRN2", target_bir_lowering=False, debug=False,
                   num_devices=NCORES)

    x_d = nc.dram_tensor("x", [CL, 128, HK, W], DT, kind="ExternalInput")
    ahn_d = nc.dram_tensor("ahn", [128, HK, NRP], DT, kind="ExternalInput")
    awn_d = nc.dram_tensor("awn", [128, WK, NCP], DT, kind="ExternalInput")
    awtb_d = nc.dram_tensor("awtb", [128, QBn, W], DT, kind="ExternalInput")
    ahtb_d = nc.dram_tensor("ahtb", [128, RBn, H], DT, kind="ExternalInput")
    o_d = nc.dram_tensor("o", [CL, 128, HK, W], DT, kind="ExternalOutput")

    with tile.TileContext(nc) as tc:
        with (
            tc.tile_pool(name="const", bufs=1) as constp,
            tc.tile_pool(name="xp", bufs=3) as xp,
            tc.tile_pool(name="rp", bufs=2) as rp,
            tc.tile_pool(name="sp", bufs=2) as sp,
            tc.tile_pool(name="up", bufs=2) as up,
            tc.tile_pool(name="op", bufs=2) as op_,
            # 8 PSUM banks total: psr 2 + pss 1 + psb 5 (stages 3+4 share)
            tc.tile_pool(name="psr", bufs=2, space="PSUM") as psr,
            tc.tile_pool(name="pss", bufs=1, space="PSUM") as pss,
            tc.tile_pool(name="psb", bufs=5, space="PSUM") as psb,
        ):
            # first channel's input in small pieces before the big broadcast
            # constants so the PE can start as early as possible (ahn is all
            # stage 1 needs); pieces land on different DMA queues and overlap
            xc0 = xp.tile([128, HK, W], DT)
            nc.sync.dma_start(xc0[:, 0:2, :], x_d[0][:, 0:2, :])
            ahn = constp.tile([128, HK, NRP], DT)
            nc.sync.dma_start(ahn[:], ahn_d[:])
            nc.sync.dma_start(xc0[:, 2:4, :], x_d[0][:, 2:4, :])
            nc.sync.dma_start(xc0[:, 4:6, :], x_d[0][:, 4:6, :])
            awn = constp.tile([128, WK, NCP], DT)
            nc.sync.dma_start(awn[:], awn_d[:])
            awtb = constp.tile([128, QBn, W], DT)
            nc.sync.dma_start(awtb[:], awtb_d[:])
            ahtb = constp.tile([128, RBn, H], DT)
            nc.sync.dma_start(ahtb[:], ahtb_d[:])

            # PSUM->SBUF casts alternate between Vector and Act so neither
            # bottlenecks (GpSimd cannot read PSUM on TRN2)
            def copier(eng):
                if eng % 2 == 0:
                    return nc.vector.tensor_copy
                return nc.scalar.copy

            # two stage-1 results fit one 2KB PSUM bank when NRP <= 256
            pair1 = 2 * NRP * 4 <= 2048

            xcs = [xc0] + [None] * (CL - 1)
            rcs = [None] * CL
            scs = [None] * CL
            ucs = [None] * CL

            def s1(c):
                """R^T[w, r] per W-chunk m (contract H in 6 chunks)."""
                xc = xcs[c]
                rc = rp.tile([128, WK, NRP], DT, tag="rc", name="rc")
                rcs[c] = rc
                if pair1:
                    for mp in range(WK // 2):
                        pr = psr.tile([128, 2, NRP], F32, tag="pr", name="pr")
                        for half in range(2):
                            m = 2 * mp + half
                            for k in range(HK):
                                nc.tensor.matmul(
                                    pr[:, half, :],
                                    xc[:, k, 128 * m:128 * m + 128],
                                    ahn[:, k, :],
                                    start=(k == 0), stop=(k == HK - 1),
                                )
                        copier(mp)(rc[:, 2 * mp:2 * mp + 2, :], pr[:])
                else:
                    for m in range(WK):
                        pr = psr.tile([128, NRP], F32, tag="pr", name="pr")
                        for k in range(HK):
                            nc.tensor.matmul(
                                pr[:],
                                xc[:, k, 128 * m:128 * m + 128],
                                ahn[:, k, :],
                                start=(k == 0), stop=(k == HK - 1),
                            )
                        copier(m)(rc[:, m, :], pr[:])

            def s2(c):
                """S^T[q, r] per q-block (contract W in 6 chunks); 1/n_q is
                baked into awn so the copy-out is a plain cast."""
                rc = rcs[c]
                sc = sp.tile([128, QBn, NRP], DT, tag="sc", name="sc")
                scs[c] = sc
                for b, (qo, qs) in enumerate(qblocks):
                    ps = pss.tile([128, NRP], F32, tag="ps", name="ps")
                    for k in range(WK):
                        nc.tensor.matmul(
                            ps[0:qs, :],
                            awn[:, k, qo:qo + qs],
                            rc[:, k, :],
                            start=(k == 0), stop=(k == WK - 1),
                        )
                    copier(b)(sc[0:qs, b, :], ps[0:qs, :])

            def s3(c):
                """U[r, j] = S^T[col_ids(j), r] per adapted r-block."""
                sc = scs[c]
                uc = up.tile([128, RBn, W], DT, tag="uc", name="uc")
                ucs[c] = uc
                for ri, (ro, rs) in enumerate(rblocks):
                    for n in range(W // NB):
                        pu = psb.tile([128, NB], F32, tag="pb", name="pu")
                        ks = qassign[n]
                        for j, b in enumerate(ks):
                            qo, qs = qblocks[b]
                            nc.tensor.matmul(
                                pu[0:rs, :],
                                sc[0:qs, b, ro:ro + rs],
                                awtb[0:qs, b, NB * n:NB * n + NB],
                                start=(j == 0), stop=(j == len(ks) - 1),
                            )
                        copier(2 * ri + n)(uc[0:rs, ri, NB * n:NB * n + NB],
                                           pu[0:rs, :])

            def s4(c):
                """OUT[i, j] via row gather; output DMA'd in 2-chunk pieces."""
                uc = ucs[c]
                ocC = op_.tile([128, HK, W], DT, tag="oc", name="ocC")
                for m in range(HK):
                    ks = rassign[m]
                    for n in range(W // NB):
                        po = psb.tile([128, NB], F32, tag="pb", name="po")
                        for j, b in enumerate(ks):
                            ro, rs = rblocks[b]
                            nc.tensor.matmul(
                                po[:],
                                ahtb[0:rs, b, 128 * m:128 * m + 128],
                                uc[0:rs, b, NB * n:NB * n + NB],
                                start=(j == 0), stop=(j == len(ks) - 1),
                            )
                        copier(2 * m + n)(ocC[:, m, NB * n:NB * n + NB],
                                          po[:])
                    if m % 2 == 1 and m < HK - 1:
                        nc.sync.dma_start(o_d[c][:, m - 1:m + 1, :],
                                          ocC[:, m - 1:m + 1, :])
                nc.sync.dma_start(o_d[c][:, HK - 2:HK, :],
                                  ocC[:, HK - 2:HK, :])

            # software-pipelined channel loop: the PE consumes each stage's
            # PSUM->SBUF casts ~3us after they were issued, so it never stalls
            for c in range(CL + 1):
                if c < CL:
                    if c + 1 < CL:
                        xn = xp.tile([128, HK, W], DT, tag="xc", name="xn")
                        nc.sync.dma_start(xn[:], x_d[c + 1])
                        xcs[c + 1] = xn
                    s1(c)
                if c >= 1:
                    s3(c - 1)
                    s4(c - 1)
                if c < CL:
                    s2(c)

    nc.compile()
    return nc


def _get_program(key):
    if key not in _cached:
        _cached[key] = _build_program(key)
    return _cached[key]


def _prepare(input, h_mask, v_mask):
    x = np.asarray(input, dtype=np.float32)
    hm = np.asarray(h_mask, dtype=np.int32)
    vm = np.asarray(v_mask, dtype=np.int32)
    assert x.shape == (1, H, W, C), x.shape

    row_ids = _segment_ids(hm[0])
    col_ids = _segment_ids(vm[0])
    nr = int(row_ids[-1]) + 1
    ncs = int(col_ids[-1]) + 1
    NRP = ((nr + 63) // 64) * 64
    NCP = ((ncs + 63) // 64) * 64

    rblocks, rassign = _adapt_blocks(row_ids, 128, nr)
    qblocks, qassign = _adapt_blocks(col_ids, NB, ncs)
    key = (NRP, NCP,
           tuple(rblocks), tuple(tuple(a) for a in rassign),
           tuple(qblocks), tuple(tuple(a) for a in qassign))

    n_r = np.bincount(row_ids, minlength=NRP).astype(np.float64)
    n_q = np.bincount(col_ids, minlength=NCP).astype(np.float64)

    # stage-1/2 weights: one-hot scaled by 1/n_r and 1/n_q (separable mean)
    ah = np.zeros((H, NRP), np.float32)
    ah[np.arange(H), row_ids] = (1.0 / n_r[row_ids])
    aw = np.zeros((W, NCP), np.float32)
    aw[np.arange(W), col_ids] = (1.0 / n_q[col_ids])
    # stage-3/4 gathers: exact one-hot
    ah1 = np.zeros((H, NRP), np.float32)
    ah1[np.arange(H), row_ids] = 1.0
    aw1 = np.zeros((W, NCP), np.float32)
    aw1[np.arange(W), col_ids] = 1.0

    # per-adapted-block partition layouts (zero padded to 128 partitions)
    QBn, RBn = len(qblocks), len(rblocks)
    awtb_dev = np.zeros((128, QBn, W), np.float32)
    for b, (qo, qs) in enumerate(qblocks):
        awtb_dev[0:qs, b, :] = aw1.T[qo:qo + qs]
    ahtb_dev = np.zeros((128, RBn, H), np.float32)
    for b, (ro, rs) in enumerate(rblocks):
        ahtb_dev[0:rs, b, :] = ah1.T[ro:ro + rs]

    ahn_dev = np.ascontiguousarray(
        ah.reshape(HK, 128, NRP).transpose(1, 0, 2)).astype(NPDT)
    awn_dev = np.ascontiguousarray(
        aw.reshape(WK, 128, NCP).transpose(1, 0, 2)).astype(NPDT)
    awtb_dev = awtb_dev.astype(NPDT)
    ahtb_dev = ahtb_dev.astype(NPDT)

    # per-core planar input: [CL, 128(p), HK(h0), W] with h = 128*h0 + p
    x64 = x[0].transpose(2, 0, 1)  # [C, H, W]
    in_maps = []
    for core in range(NCORES):
        xc = x64[CL * core:CL * (core + 1)]  # [CL, H, W]
        xdev = np.ascontiguousarray(
            xc.reshape(CL, HK, 128, W).transpose(0, 2, 1, 3)).astype(NPDT)
        in_maps.append({
            "x": xdev,
            "ahn": ahn_dev,
            "awn": awn_dev,
            "awtb": awtb_dev,
            "ahtb": ahtb_dev,
        })
    return in_maps, key


def _assemble(results):
    out = np.empty((1, H, W, C), np.float32)
    for core in range(NCORES):
        o = np.asarray(results[core]["o"]).astype(np.float32)  # [CL,128,HK,W]
        oc = o.transpose(0, 2, 1, 3).reshape(CL, H, W)         # h = 128*m + p
        out[0, :, :, CL * core:CL * (core + 1)] = oc.transpose(1, 2, 0)
    return out


def run(inputs: dict, trace: bool = False, **kwargs):
    """Full pipeline; returns (output, BassKernelResults)."""
    in_maps, key = _prepare(**inputs)
    nc = _get_program(key)
    res = run_bass_kernel_spmd(nc, in_maps, list(range(NCORES)),
                               trace=trace, **kwargs)
    return _assemble(res.results), res


def kernel(**inputs) -> np.ndarray:
    out, _ = run(inputs, trace=False)
    return out


# revision 8
# speedup vs baseline: 1.0431x; 1.0071x over previous
"""Grid pooling (segment mean over rectangular grid cells) on 8 trn2 cores.

Math: row/col masks induce contiguous run-segments along H and W, so every
grid cell is a rectangle and the whole op factorizes per channel as

    out_c = A_h @ ( A_h'^T @ X_c @ A_w' ) @ A_w^T

with segment-assignment matrices built on host from the tiny masks. The
mean scale 1/(n_r*n_q) is separable: 1/n_r is baked into the stage-1
weights A_h' and 1/n_q into the stage-2 weights A_w' (bf16), so every
PSUM->SBUF move is a plain cast. Channels (64) are sharded 8-way across
cores; each core runs 8 independent 768x768 channel planes through 4
matmul stages:

  1. R^T  = X_c^T  @ A_h'   (row-segment sums;   lhsT = X chunks)
  2. S^T  = A_w'^T @ R^T    (col-segment means;  lhsT = A_w' chunks)
  3. U    = S^T-gather cols (adapted r-blocks, single-k matmuls)
  4. OUT  = A_h-gather rows (per h-chunk)

Stage-1/2 matmuls stream ~192 columns, which exactly hides the ~75ns
LDWEIGHTS of each 128-row weight load and keeps the PE p-state at full
clock. The channel loop is software-pipelined as

    s1(c) -> s3(c-1) -> s4(c-1) -> s2(c)

so the PE never waits for a PSUM->SBUF cast to finish (each stage's
consumer runs ~3us after its producer). Casts alternate between the
Vector and Act engines (GpSimd cannot read PSUM on TRN2); stages 3+4
share one 5-deep PSUM pool so casts never backpressure the PE.
"""

import numpy as np
import ml_dtypes

from concourse import bacc, tile
import concourse.mybir as mybir
from concourse.bass_utils import run_bass_kernel_spmd

H = 768
W = 768
C = 64
NCORES = 8
CL = C // NCORES          # channels per core
HK = H // 128             # 6 H-chunks (contraction / output chunks)
WK = W // 128             # 6 W-chunks
NB = 384                  # free-dim tile for broadcast stages (768 = 2*384)

DT = mybir.dt.bfloat16
F32 = mybir.dt.float32
NPDT = ml_dtypes.bfloat16

_cached = {}


def _segment_ids(mask: np.ndarray) -> np.ndarray:
    """mask [L] binary -> segment ids via rising edges (pixel 0 -> seg 0)."""
    m = mask.astype(np.int64)
    prev = np.concatenate([[0], m[:-1]])
    rising = (m == 1) & (prev == 0)
    rising[0] = False
    return np.cumsum(rising.astype(np.int64)).astype(np.int32)


def _adapt_blocks(ids: np.ndarray, tile_len: int, nseg: int):
    """Partition the segment axis into blocks of <=128 ids such that the id
    range of every output tile of `tile_len` positions lies in one block.

    Returns (blocks, assign): blocks = [(start, width), ...] (may overlap by
    a shared boundary segment), assign[t] = [(block_idx), ...] the block(s)
    tile t accumulates over. Single-element lists on the fast path; falls
    back to fixed disjoint 128-blocks (multi-k) if a range is too wide.
    """
    L = len(ids)
    ntiles = L // tile_len
    ranges = [(int(ids[t * tile_len]), int(ids[(t + 1) * tile_len - 1]))
              for t in range(ntiles)]
    if all(hi - lo + 1 <= 128 for lo, hi in ranges):
        blocks, assign = [], []
        cur_start, cur_end = 0, -1
        for lo, hi in ranges:
            if hi - cur_start + 1 <= 128 and cur_end >= 0:
                cur_end = max(cur_end, hi)
            else:
                if cur_end >= 0:
                    blocks.append((cur_start, cur_end - cur_start + 1))
                cur_start, cur_end = lo, hi
            assign.append(len(blocks))
        blocks.append((cur_start, cur_end - cur_start + 1))
        return blocks, [[a] for a in assign]
    blocks = [(off, min(128, nseg - off)) for off in range(0, nseg, 128)]
    assign = []
    for lo, hi in ranges:
        assign.append([b for b, (s, w) in enumerate(blocks)
                       if not (hi < s or lo > s + w - 1)])
    return blocks, assign


def _build_program(key):
    (NRP, NCP, rblocks, rassign, qblocks, qassign) = key
    RBn = len(rblocks)
    QBn = len(qblocks)

    nc = bacc.Bacc("TRN2", target_bir_lowering=False, debug=False,
                   num_devices=NCORES)

    x_d = nc.dram_tensor("x", [CL, 128, HK, W], DT, kind="ExternalInput")
    ahn_d = nc.dram_tensor("ahn", [128, HK, NRP], DT, kind="ExternalInput")
    awn_d = nc.dram_tensor("awn", [128, WK, NCP], DT, kind="ExternalInput")
    awtb_d = nc.dram_tensor("awtb", [128, QBn, W], DT, kind="ExternalInput")
    ahtb_d = nc.dram_tensor("ahtb", [128, RBn, H], DT, kind="ExternalInput")
    o_d = nc.dram_tensor("o", [CL, 128, HK, W], DT, kind="ExternalOutput")

    with tile.TileContext(nc) as tc:
        with (
            tc.tile_pool(name="const", bufs=1) as constp,
            tc.tile_pool(name="xp", bufs=3) as xp,
            tc.tile_pool(name="rp", bufs=2) as rp,
            tc.tile_pool(name="sp", bufs=2) as sp,
            tc.tile_pool(name="up", bufs=2) as up,
            tc.tile_pool(name="op", bufs=2) as op_,
            # 8 PSUM banks total: psr 2 + pss 1 + psb 5 (stages 3+4 share)
            tc.tile_pool(name="psr", bufs=2, space="PSUM") as psr,
            tc.tile_pool(name="pss", bufs=1, space="PSUM") as pss,
            tc.tile_pool(name="psb", bufs=5, space="PSUM") as psb,
        ):
            # first channel's input in small pieces before the big broadcast
            # constants so the PE can start as early as possible (ahn is all
            # stage 1 needs); pieces land on different DMA queues and overlap
            xc0 = xp.tile([128, HK, W], DT)
            for k in range(HK):
                nc.sync.dma_start(xc0[:, k:k + 1, :], x_d[0][:, k:k + 1, :])
            ahn = constp.tile([128, HK, NRP], DT)
            nc.sync.dma_start(ahn[:], ahn_d[:])
            awn = constp.tile([128, WK, NCP], DT)
            nc.sync.dma_start(awn[:], awn_d[:])
            awtb = constp.tile([128, QBn, W], DT)
            nc.sync.dma_start(awtb[:], awtb_d[:])
            ahtb = constp.tile([128, RBn, H], DT)
            nc.sync.dma_start(ahtb[:], ahtb_d[:])

            # PSUM->SBUF casts alternate between Vector and Act so neither
            # bottlenecks (GpSimd cannot read PSUM on TRN2)
            def copier(eng):
                if eng % 2 == 0:
                    return nc.vector.tensor_copy
                return nc.scalar.copy

            # two stage-1 results fit one 2KB PSUM bank when NRP <= 256
            pair1 = 2 * NRP * 4 <= 2048

            xcs = [xc0] + [None] * (CL - 1)
            rcs = [None] * CL
            scs = [None] * CL
            ucs = [None] * CL

            def s1(c):
                """R^T[w, r] per W-chunk m (contract H in 6 chunks)."""
                xc = xcs[c]
                rc = rp.tile([128, WK, NRP], DT, tag="rc", name="rc")
                rcs[c] = rc
                if pair1:
                    for mp in range(WK // 2):
                        pr = psr.tile([128, 2, NRP], F32, tag="pr", name="pr")
                        for half in range(2):
                            m = 2 * mp + half
                            for k in range(HK):
                                nc.tensor.matmul(
                                    pr[:, half, :],
                                    xc[:, k, 128 * m:128 * m + 128],
                                    ahn[:, k, :],
                                    start=(k == 0), stop=(k == HK - 1),
                                )
                        copier(mp)(rc[:, 2 * mp:2 * mp + 2, :], pr[:])
                else:
                    for m in range(WK):
                        pr = psr.tile([128, NRP], F32, tag="pr", name="pr")
                        for k in range(HK):
                            nc.tensor.matmul(
                                pr[:],
                                xc[:, k, 128 * m:128 * m + 128],
                                ahn[:, k, :],
                                start=(k == 0), stop=(k == HK - 1),
                            )
                        copier(m)(rc[:, m, :], pr[:])

            def s2(c):
                """S^T[q, r] per q-block (contract W in 6 chunks); 1/n_q is
                baked into awn so the copy-out is a plain cast."""
                rc = rcs[c]
                sc = sp.tile([128, QBn, NRP], DT, tag="sc", name="sc")
                scs[c] = sc
                for b, (qo, qs) in enumerate(qblocks):
                    ps = pss.tile([128, NRP], F32, tag="ps", name="ps")
                    for k in range(WK):
                        nc.tensor.matmul(
                            ps[0:qs, :],
                            awn[:, k, qo:qo + qs],
                            rc[:, k, :],
                            start=(k == 0), stop=(k == WK - 1),
                        )
                    copier(b)(sc[0:qs, b, :], ps[0:qs, :])

            def s3(c):
                """U[r, j] = S^T[col_ids(j), r] per adapted r-block."""
                sc = scs[c]
                uc = up.tile([128, RBn, W], DT, tag="uc", name="uc")
                ucs[c] = uc
                for ri, (ro, rs) in enumerate(rblocks):
                    for n in range(W // NB):
                        pu = psb.tile([128, NB], F32, tag="pb", name="pu")
                        ks = qassign[n]
                        for j, b in enumerate(ks):
                            qo, qs = qblocks[b]
                            nc.tensor.matmul(
                                pu[0:rs, :],
                                sc[0:qs, b, ro:ro + rs],
                                awtb[0:qs, b, NB * n:NB * n + NB],
                                start=(j == 0), stop=(j == len(ks) - 1),
                            )
                        copier(2 * ri + n)(uc[0:rs, ri, NB * n:NB * n + NB],
                                           pu[0:rs, :])

            def s4(c):
                """OUT[i, j] via row gather; output DMA'd in 2-chunk pieces."""
                uc = ucs[c]
                ocC = op_.tile([128, HK, W], DT, tag="oc", name="ocC")
                for m in range(HK):
                    ks = rassign[m]
                    for n in range(W // NB):
                        po = psb.tile([128, NB], F32, tag="pb", name="po")
                        for j, b in enumerate(ks):
                            ro, rs = rblocks[b]
                            nc.tensor.matmul(
                                po[:],
                                ahtb[0:rs, b, 128 * m:128 * m + 128],
                                uc[0:rs, b, NB * n:NB * n + NB],
                                start=(j == 0), stop=(j == len(ks) - 1),
                            )
                        copier(2 * m + n)(ocC[:, m, NB * n:NB * n + NB],
                                          po[:])
                    if c == CL - 1:
                        nc.sync.dma_start(o_d[c][:, m:m + 1, :],
                                          ocC[:, m:m + 1, :])
                    elif m % 2 == 1:
                        nc.sync.dma_start(o_d[c][:, m - 1:m + 1, :],
                                          ocC[:, m - 1:m + 1, :])

            # software-pipelined channel loop: the PE consumes each stage's
            # PSUM->SBUF casts ~3us after they were issued, so it never stalls
            for c in range(CL + 1):
                if c < CL:
                    s1(c)
                    if c + 1 < CL:
                        xn = xp.tile([128, HK, W], DT, tag="xc", name="xn")
                        nc.sync.dma_start(xn[:], x_d[c + 1])
                        xcs[c + 1] = xn
                if c >= 1:
                    s3(c - 1)
                    s4(c - 1)
                if c < CL:
                    s2(c)

    nc.compile()
    return nc


def _get_program(key):
    if key not in _cached:
        _cached[key] = _build_program(key)
    return _cached[key]


def _prepare(input, h_mask, v_mask):
    x = np.asarray(input, dtype=np.float32)
    hm = np.asarray(h_mask, dtype=np.int32)
    vm = np.asarray(v_mask, dtype=np.int32)
    assert x.shape == (1, H, W, C), x.shape

    row_ids = _segment_ids(hm[0])
    col_ids = _segment_ids(vm[0])
    nr = int(row_ids[-1]) + 1
    ncs = int(col_ids[-1]) + 1
    NRP = ((nr + 63) // 64) * 64
    NCP = ((ncs + 63) // 64) * 64

    rblocks, rassign = _adapt_blocks(row_ids, 128, nr)
    qblocks, qassign = _adapt_blocks(col_ids, NB, ncs)
    key = (NRP, NCP,
           tuple(rblocks), tuple(tuple(a) for a in rassign),
           tuple(qblocks), tuple(tuple(a) for a in qassign))

    n_r = np.bincount(row_ids, minlength=NRP).astype(np.float64)
    n_q = np.bincount(col_ids, minlength=NCP).astype(np.float64)

    # stage-1/2 weights: one-hot scaled by 1/n_r and 1/n_q (separable mean)
    ah = np.zeros((H, NRP), np.float32)
    ah[np.arange(H), row_ids] = (1.0 / n_r[row_ids])
    aw = np.zeros((W, NCP), np.float32)
    aw[np.arange(W), col_ids] = (1.0 / n_q[col_ids])
    # stage-3/4 gathers: exact one-hot
    ah1 = np.zeros((H, NRP), np.float32)
    ah1[np.arange(H), row_ids] = 1.0
    aw1 = np.zeros((W, NCP), np.float32)
    aw1[np.arange(W), col_ids] = 1.0

    # per-adapted-block partition layouts (zero padded to 128 partitions)
    QBn, RBn = len(qblocks), len(rblocks)
    awtb_dev = np.zeros((128, QBn, W), np.float32)
    for b, (qo, qs) in enumerate(qblocks):
        awtb_dev[0:qs, b, :] = aw1.T[qo:qo + qs]
    ahtb_dev = np.zeros((128, RBn, H), np.float32)
    for b, (ro, rs) in enumerate(rblocks):
        ahtb_dev[0:rs, b, :] = ah1.T[ro:ro + rs]

    ahn_dev = np.ascontiguousarray(
        ah.reshape(HK, 128, NRP).transpose(1, 0, 2)).astype(NPDT)
    awn_dev = np.ascontiguousarray(
        aw.reshape(WK, 128, NCP).transpose(1, 0, 2)).astype(NPDT)
    awtb_dev = awtb_dev.astype(NPDT)
    ahtb_dev = ahtb_dev.astype(NPDT)

    # per-core planar input: [CL, 128(p), HK(h0), W] with h = 128*h0 + p
    x64 = x[0].transpose(2, 0, 1)  # [C, H, W]
    in_maps = []
    for core in range(NCORES):
        xc = x64[CL * core:CL * (core + 1)]  # [CL, H, W]
        xdev = np.ascontiguousarray(
            xc.reshape(CL, HK, 128, W).transpose(0, 2, 1, 3)).astype(NPDT)
        in_maps.append({
            "x": xdev,
            "ahn": ahn_dev,
            "awn": awn_dev,
            "awtb": awtb_dev,
            "ahtb": ahtb_dev,
        })
    return in_maps, key


def _assemble(results):
    out = np.empty((1, H, W, C), np.float32)
    for core in range(NCORES):
        o = np.asarray(results[core]["o"]).astype(np.float32)  # [CL,128,HK,W]
        oc = o.transpose(0, 2, 1, 3).reshape(CL, H, W)         # h = 128*m + p
        out[0, :, :, CL * core:CL * (core + 1)] = oc.transpose(1, 2, 0)
    return out


def run(inputs: dict, trace: bool = False, **kwargs):
    """Full pipeline; returns (output, BassKernelResults)."""
    in_maps, key = _prepare(**inputs)
    nc = _get_program(key)
    res = run_bass_kernel_spmd(nc, in_maps, list(range(NCORES)),
                               trace=trace, **kwargs)
    return _assemble(res.results), res


def kernel(**inputs) -> np.ndarray:
    out, _ = run(inputs, trace=False)
    return out


# revision 10
# speedup vs baseline: 1.0465x; 1.0033x over previous
"""Grid pooling (segment mean over rectangular grid cells) on 8 trn2 cores.

Math: row/col masks induce contiguous run-segments along H and W, so every
grid cell is a rectangle and the whole op factorizes per channel as

    out_c = A_h @ ( A_h'^T @ X_c @ A_w' ) @ A_w^T

with segment-assignment matrices built on host from the tiny masks. The
mean scale 1/(n_r*n_q) is separable: 1/n_r is baked into the stage-1
weights A_h' and 1/n_q into the stage-2 weights A_w' (bf16), so every
PSUM->SBUF move is a plain cast. Channels (64) are sharded 8-way across
cores; each core runs 8 independent 768x768 channel planes through 4
matmul stages:

  1. R^T  = X_c^T  @ A_h'   (row-segment sums;   lhsT = X chunks)
  2. S^T  = A_w'^T @ R^T    (col-segment means;  lhsT = A_w' chunks)
  3. U    = S^T-gather cols (adapted r-blocks, single-k matmuls)
  4. OUT  = A_h-gather rows (per h-chunk)

Stage-1/2 matmuls stream ~192 columns, which exactly hides the ~75ns
LDWEIGHTS of each 128-row weight load and keeps the PE p-state at full
clock. The channel loop is software-pipelined as

    s1(c) -> s3(c-1) -> s4(c-1) -> s2(c)

so the PE never waits for a PSUM->SBUF cast to finish (each stage's
consumer runs ~3us after its producer). Casts alternate between the
Vector and Act engines (GpSimd cannot read PSUM on TRN2); stages 3+4
share one 5-deep PSUM pool so casts never backpressure the PE.
"""

import numpy as np
import ml_dtypes

from concourse import bacc, tile
import concourse.mybir as mybir
from concourse.bass_utils import run_bass_kernel_spmd

H = 768
W = 768
C = 64
NCORES = 8
CL = C // NCORES          # channels per core
HK = H // 128             # 6 H-chunks (contraction / output chunks)
WK = W // 128             # 6 W-chunks
NB = 384                  # free-dim tile for broadcast stages (768 = 2*384)

DT = mybir.dt.bfloat16
F32 = mybir.dt.float32
NPDT = ml_dtypes.bfloat16

_cached = {}


def _segment_ids(mask: np.ndarray) -> np.ndarray:
    """mask [L] binary -> segment ids via rising edges (pixel 0 -> seg 0)."""
    m = mask.astype(np.int64)
    prev = np.concatenate([[0], m[:-1]])
    rising = (m == 1) & (prev == 0)
    rising[0] = False
    return np.cumsum(rising.astype(np.int64)).astype(np.int32)


def _adapt_blocks(ids: np.ndarray, tile_len: int, nseg: int):
    """Partition the segment axis into blocks of <=128 ids such that the id
    range of every output tile of `tile_len` positions lies in one block.

    Returns (blocks, assign): blocks = [(start, width), ...] (may overlap by
    a shared boundary segment), assign[t] = [(block_idx), ...] the block(s)
    tile t accumulates over. Single-element lists on the fast path; falls
    back to fixed disjoint 128-blocks (multi-k) if a range is too wide.
    """
    L = len(ids)
    ntiles = L // tile_len
    ranges = [(int(ids[t * tile_len]), int(ids[(t + 1) * tile_len - 1]))
              for t in range(ntiles)]
    if all(hi - lo + 1 <= 128 for lo, hi in ranges):
        blocks, assign = [], []
        cur_start, cur_end = 0, -1
        for lo, hi in ranges:
            if hi - cur_start + 1 <= 128 and cur_end >= 0:
                cur_end = max(cur_end, hi)
            else:
                if cur_end >= 0:
                    blocks.append((cur_start, cur_end - cur_start + 1))
                cur_start, cur_end = lo, hi
            assign.append(len(blocks))
        blocks.append((cur_start, cur_end - cur_start + 1))
        return blocks, [[a] for a in assign]
    blocks = [(off, min(128, nseg - off)) for off in range(0, nseg, 128)]
    assign = []
    for lo, hi in ranges:
        assign.append([b for b, (s, w) in enumerate(blocks)
                       if not (hi < s or lo > s + w - 1)])
    return blocks, assign


def _build_program(key):
    (NRP, NCP, rblocks, rassign, qblocks, qassign) = key
    RBn = len(rblocks)
    QBn = len(qblocks)

    nc = bacc.Bacc("TRN2", target_bir_lowering=False, debug=False,
                   num_devices=NCORES)

    x_d = nc.dram_tensor("x", [CL, 128, HK, W], DT, kind="ExternalInput")
    ahn_d = nc.dram_tensor("ahn", [128, HK, NRP], DT, kind="ExternalInput")
    awn_d = nc.dram_tensor("awn", [128, WK, NCP], DT, kind="ExternalInput")
    awtb_d = nc.dram_tensor("awtb", [128, QBn, W], DT, kind="ExternalInput")
    ahtb_d = nc.dram_tensor("ahtb", [128, RBn, H], DT, kind="ExternalInput")
    o_d = nc.dram_tensor("o", [CL, 128, HK, W], DT, kind="ExternalOutput")

    with tile.TileContext(nc) as tc:
        with (
            tc.tile_pool(name="const", bufs=1) as constp,
            tc.tile_pool(name="xp", bufs=3) as xp,
            tc.tile_pool(name="rp", bufs=2) as rp,
            tc.tile_pool(name="sp", bufs=2) as sp,
            tc.tile_pool(name="up", bufs=2) as up,
            tc.tile_pool(name="op", bufs=2) as op_,
            # 8 PSUM banks total: psr 2 + pss 1 + psb 5 (stages 3+4 share)
            tc.tile_pool(name="psr", bufs=2, space="PSUM") as psr,
            tc.tile_pool(name="pss", bufs=1, space="PSUM") as pss,
            tc.tile_pool(name="psb", bufs=5, space="PSUM") as psb,
        ):
            # first channel's input in small pieces before the big broadcast
            # constants so the PE can start as early as possible (ahn is all
            # stage 1 needs); pieces land on different DMA queues and overlap
            xc0 = xp.tile([128, HK, W], DT)
            nc.sync.dma_start(xc0[:, 0:1, :], x_d[0][:, 0:1, :])
            ahn = constp.tile([128, HK, NRP], DT)
            nc.sync.dma_start(ahn[:], ahn_d[:])
            for k in range(1, HK):
                nc.sync.dma_start(xc0[:, k:k + 1, :], x_d[0][:, k:k + 1, :])
            awn = constp.tile([128, WK, NCP], DT)
            nc.sync.dma_start(awn[:], awn_d[:])
            awtb = constp.tile([128, QBn, W], DT)
            nc.sync.dma_start(awtb[:], awtb_d[:])
            ahtb = constp.tile([128, RBn, H], DT)
            nc.sync.dma_start(ahtb[:], ahtb_d[:])

            # PSUM->SBUF casts alternate between Vector and Act so neither
            # bottlenecks (GpSimd cannot read PSUM on TRN2)
            def copier(eng):
                if eng % 2 == 0:
                    return nc.vector.tensor_copy
                return nc.scalar.copy

            # two stage-1 results fit one 2KB PSUM bank when NRP <= 256
            pair1 = 2 * NRP * 4 <= 2048

            xcs = [xc0] + [None] * (CL - 1)
            rcs = [None] * CL
            scs = [None] * CL
            ucs = [None] * CL

            def s1(c):
                """R^T[w, r] per W-chunk m (contract H in 6 chunks)."""
                xc = xcs[c]
                rc = rp.tile([128, WK, NRP], DT, tag="rc", name="rc")
                rcs[c] = rc
                if pair1:
                    for mp in range(WK // 2):
                        pr = psr.tile([128, 2, NRP], F32, tag="pr", name="pr")
                        for half in range(2):
                            m = 2 * mp + half
                            for k in range(HK):
                                nc.tensor.matmul(
                                    pr[:, half, :],
                                    xc[:, k, 128 * m:128 * m + 128],
                                    ahn[:, k, :],
                                    start=(k == 0), stop=(k == HK - 1),
                                )
                        copier(mp)(rc[:, 2 * mp:2 * mp + 2, :], pr[:])
                else:
                    for m in range(WK):
                        pr = psr.tile([128, NRP], F32, tag="pr", name="pr")
                        for k in range(HK):
                            nc.tensor.matmul(
                                pr[:],
                                xc[:, k, 128 * m:128 * m + 128],
                                ahn[:, k, :],
                                start=(k == 0), stop=(k == HK - 1),
                            )
                        copier(m)(rc[:, m, :], pr[:])

            def s2(c):
                """S^T[q, r] per q-block (contract W in 6 chunks); 1/n_q is
                baked into awn so the copy-out is a plain cast."""
                rc = rcs[c]
                sc = sp.tile([128, QBn, NRP], DT, tag="sc", name="sc")
                scs[c] = sc
                for b, (qo, qs) in enumerate(qblocks):
                    ps = pss.tile([128, NRP], F32, tag="ps", name="ps")
                    for k in range(WK):
                        nc.tensor.matmul(
                            ps[0:qs, :],
                            awn[:, k, qo:qo + qs],
                            rc[:, k, :],
                            start=(k == 0), stop=(k == WK - 1),
                        )
                    copier(b)(sc[0:qs, b, :], ps[0:qs, :])

            def s3(c):
                """U[r, j] = S^T[col_ids(j), r] per adapted r-block."""
                sc = scs[c]
                uc = up.tile([128, RBn, W], DT, tag="uc", name="uc")
                ucs[c] = uc
                for ri, (ro, rs) in enumerate(rblocks):
                    for n in range(W // NB):
                        pu = psb.tile([128, NB], F32, tag="pb", name="pu")
                        ks = qassign[n]
                        for j, b in enumerate(ks):
                            qo, qs = qblocks[b]
                            nc.tensor.matmul(
                                pu[0:rs, :],
                                sc[0:qs, b, ro:ro + rs],
                                awtb[0:qs, b, NB * n:NB * n + NB],
                                start=(j == 0), stop=(j == len(ks) - 1),
                            )
                        copier(2 * ri + n)(uc[0:rs, ri, NB * n:NB * n + NB],
                                           pu[0:rs, :])

            def s4(c):
                """OUT[i, j] via row gather; output DMA'd in 2-chunk pieces."""
                uc = ucs[c]
                ocC = op_.tile([128, HK, W], DT, tag="oc", name="ocC")
                for m in range(HK):
                    ks = rassign[m]
                    for n in range(W // NB):
                        po = psb.tile([128, NB], F32, tag="pb", name="po")
                        for j, b in enumerate(ks):
                            ro, rs = rblocks[b]
                            nc.tensor.matmul(
                                po[:],
                                ahtb[0:rs, b, 128 * m:128 * m + 128],
                                uc[0:rs, b, NB * n:NB * n + NB],
                                start=(j == 0), stop=(j == len(ks) - 1),
                            )
                        copier(2 * m + n)(ocC[:, m, NB * n:NB * n + NB],
                                          po[:])
                    if c == CL - 1:
                        nc.sync.dma_start(o_d[c][:, m:m + 1, :],
                                          ocC[:, m:m + 1, :])
                    elif m % 2 == 1:
                        nc.sync.dma_start(o_d[c][:, m - 1:m + 1, :],
                                          ocC[:, m - 1:m + 1, :])

            # software-pipelined channel loop: the PE consumes each stage's
            # PSUM->SBUF casts ~3us after they were issued, so it never stalls
            for c in range(CL + 1):
                if c < CL:
                    s1(c)
                    if c + 1 < CL:
                        xn = xp.tile([128, HK, W], DT, tag="xc", name="xn")
                        nc.sync.dma_start(xn[:], x_d[c + 1])
                        xcs[c + 1] = xn
                if c >= 1:
                    s3(c - 1)
                    s4(c - 1)
                if c < CL:
                    s2(c)

    nc.compile()
    return nc


def _get_program(key):
    if key not in _cached:
        _cached[key] = _build_program(key)
    return _cached[key]


def _prepare(input, h_mask, v_mask):
    x = np.asarray(input, dtype=np.float32)
    hm = np.asarray(h_mask, dtype=np.int32)
    vm = np.asarray(v_mask, dtype=np.int32)
    assert x.shape == (1, H, W, C), x.shape

    row_ids = _segment_ids(hm[0])
    col_ids = _segment_ids(vm[0])
    nr = int(row_ids[-1]) + 1
    ncs = int(col_ids[-1]) + 1
    NRP = ((nr + 63) // 64) * 64
    NCP = ((ncs + 63) // 64) * 64

    rblocks, rassign = _adapt_blocks(row_ids, 128, nr)
    qblocks, qassign = _adapt_blocks(col_ids, NB, ncs)

    # Pad adapted blocks to 128 ids (overlapping is fine on the single-block
    # fast path: the gather matrices' extra rows are all-zero for the tiles
    # assigned to the block). Narrow-partition matmuls (M or K = 62) measure
    # ~1.5x slower per column on hardware than full 128-wide ones.
    def _pad_blocks(blocks, assign, npad):
        if any(len(a) != 1 for a in assign):
            return blocks   # multi-block accumulation needs disjoint blocks
        wmax = min(128, npad)
        return [(min(o, npad - wmax), wmax) for o, w in blocks]

    rblocks = _pad_blocks(rblocks, rassign, NRP)
    qblocks = _pad_blocks(qblocks, qassign, NCP)
    key = (NRP, NCP,
           tuple(rblocks), tuple(tuple(a) for a in rassign),
           tuple(qblocks), tuple(tuple(a) for a in qassign))

    n_r = np.bincount(row_ids, minlength=NRP).astype(np.float64)
    n_q = np.bincount(col_ids, minlength=NCP).astype(np.float64)

    # stage-1/2 weights: one-hot scaled by 1/n_r and 1/n_q (separable mean)
    ah = np.zeros((H, NRP), np.float32)
    ah[np.arange(H), row_ids] = (1.0 / n_r[row_ids])
    aw = np.zeros((W, NCP), np.float32)
    aw[np.arange(W), col_ids] = (1.0 / n_q[col_ids])
    # stage-3/4 gathers: exact one-hot
    ah1 = np.zeros((H, NRP), np.float32)
    ah1[np.arange(H), row_ids] = 1.0
    aw1 = np.zeros((W, NCP), np.float32)
    aw1[np.arange(W), col_ids] = 1.0

    # per-adapted-block partition layouts (zero padded to 128 partitions)
    QBn, RBn = len(qblocks), len(rblocks)
    awtb_dev = np.zeros((128, QBn, W), np.float32)
    for b, (qo, qs) in enumerate(qblocks):
        awtb_dev[0:qs, b, :] = aw1.T[qo:qo + qs]
    ahtb_dev = np.zeros((128, RBn, H), np.float32)
    for b, (ro, rs) in enumerate(rblocks):
        ahtb_dev[0:rs, b, :] = ah1.T[ro:ro + rs]

    ahn_dev = np.ascontiguousarray(
        ah.reshape(HK, 128, NRP).transpose(1, 0, 2)).astype(NPDT)
    awn_dev = np.ascontiguousarray(
        aw.reshape(WK, 128, NCP).transpose(1, 0, 2)).astype(NPDT)
    awtb_dev = awtb_dev.astype(NPDT)
    ahtb_dev = ahtb_dev.astype(NPDT)

    # per-core planar input: [CL, 128(p), HK(h0), W] with h = 128*h0 + p
    x64 = x[0].transpose(2, 0, 1)  # [C, H, W]
    in_maps = []
    for core in range(NCORES):
        xc = x64[CL * core:CL * (core + 1)]  # [CL, H, W]
        xdev = np.ascontiguousarray(
            xc.reshape(CL, HK, 128, W).transpose(0, 2, 1, 3)).astype(NPDT)
        in_maps.append({
            "x": xdev,
            "ahn": ahn_dev,
            "awn": awn_dev,
            "awtb": awtb_dev,
            "ahtb": ahtb_dev,
        })
    return in_maps, key


def _assemble(results):
    out = np.empty((1, H, W, C), np.float32)
    for core in range(NCORES):
        o = np.asarray(results[core]["o"]).astype(np.float32)  # [CL,128,HK,W]
        oc = o.transpose(0, 2, 1, 3).reshape(CL, H, W)         # h = 128*m + p
        out[0, :, :, CL * core:CL * (core + 1)] = oc.transpose(1, 2, 0)
    return out


def run(inputs: dict, trace: bool = False, **kwargs):
    """Full pipeline; returns (output, BassKernelResults)."""
    in_maps, key = _prepare(**inputs)
    nc = _get_program(key)
    res = run_bass_kernel_spmd(nc, in_maps, list(range(NCORES)),
                               trace=trace, **kwargs)
    return _assemble(res.results), res


def kernel(**inputs) -> np.ndarray:
    out, _ = run(inputs, trace=False)
    return out


# revision 11
# speedup vs baseline: 1.1026x; 1.0536x over previous
"""Grid pooling (segment mean over rectangular grid cells) on 8 trn2 cores.

Math: row/col masks induce contiguous run-segments along H and W, so every
grid cell is a rectangle and the whole op factorizes per channel as

    out_c = A_h @ ( A_h'^T @ X_c @ A_w' ) @ A_w^T

with segment-assignment matrices built on host from the tiny masks. The
mean scale 1/(n_r*n_q) is separable: 1/n_r is baked into the stage-1
weights A_h' and 1/n_q into the stage-2 weights A_w' (bf16), so every
PSUM->SBUF move is a plain cast. Channels (64) are sharded 8-way across
cores; each core runs 8 independent 768x768 channel planes through 4
matmul stages:

  1. R^T  = X_c^T  @ A_h'   (row-segment sums;   lhsT = X chunks)
  2. S^T  = A_w'^T @ R^T    (col-segment means;  lhsT = A_w' chunks)
  3. U    = S^T-gather cols (adapted r-blocks, single-k matmuls)
  4. OUT  = A_h-gather rows (per h-chunk)

Stage-1/2 matmuls stream ~192 columns, which exactly hides the ~75ns
LDWEIGHTS of each 128-row weight load and keeps the PE p-state at full
clock. The channel loop is software-pipelined as

    s1(c) -> s3(c-1) -> s4(c-1) -> s2(c)

so the PE never waits for a PSUM->SBUF cast to finish (each stage's
consumer runs ~3us after its producer). Casts alternate between the
Vector and Act engines (GpSimd cannot read PSUM on TRN2); stages 3+4
share one 5-deep PSUM pool so casts never backpressure the PE.
"""

import numpy as np
import ml_dtypes

from concourse import bacc, tile
import concourse.mybir as mybir
from concourse.bass_utils import run_bass_kernel_spmd

H = 768
W = 768
C = 64
NCORES = 8
CL = C // NCORES          # channels per core
HK = H // 128             # 6 H-chunks (contraction / output chunks)
WK = W // 128             # 6 W-chunks
NB = 384                  # free-dim tile for broadcast stages (768 = 2*384)

DT = mybir.dt.bfloat16
F32 = mybir.dt.float32
NPDT = ml_dtypes.bfloat16

_cached = {}


def _segment_ids(mask: np.ndarray) -> np.ndarray:
    """mask [L] binary -> segment ids via rising edges (pixel 0 -> seg 0)."""
    m = mask.astype(np.int64)
    prev = np.concatenate([[0], m[:-1]])
    rising = (m == 1) & (prev == 0)
    rising[0] = False
    return np.cumsum(rising.astype(np.int64)).astype(np.int32)


def _adapt_blocks(ids: np.ndarray, tile_len: int, nseg: int):
    """Partition the segment axis into blocks of <=128 ids such that the id
    range of every output tile of `tile_len` positions lies in one block.

    Returns (blocks, assign): blocks = [(start, width), ...] (may overlap by
    a shared boundary segment), assign[t] = [(block_idx), ...] the block(s)
    tile t accumulates over. Single-element lists on the fast path; falls
    back to fixed disjoint 128-blocks (multi-k) if a range is too wide.
    """
    L = len(ids)
    ntiles = L // tile_len
    ranges = [(int(ids[t * tile_len]), int(ids[(t + 1) * tile_len - 1]))
              for t in range(ntiles)]
    if all(hi - lo + 1 <= 128 for lo, hi in ranges):
        blocks, assign = [], []
        cur_start, cur_end = 0, -1
        for lo, hi in ranges:
            if hi - cur_start + 1 <= 128 and cur_end >= 0:
                cur_end = max(cur_end, hi)
            else:
                if cur_end >= 0:
                    blocks.append((cur_start, cur_end - cur_start + 1))
                cur_start, cur_end = lo, hi
            assign.append(len(blocks))
        blocks.append((cur_start, cur_end - cur_start + 1))
        return blocks, [[a] for a in assign]
    blocks = [(off, min(128, nseg - off)) for off in range(0, nseg, 128)]
    assign = []
    for lo, hi in ranges:
        assign.append([b for b, (s, w) in enumerate(blocks)
                       if not (hi < s or lo > s + w - 1)])
    return blocks, assign


def _build_program(key):
    (NRP, NCP, rblocks, rassign, qblocks, qassign) = key
    RBn = len(rblocks)
    QBn = len(qblocks)

    nc = bacc.Bacc("TRN2", target_bir_lowering=False, debug=False,
                   num_devices=NCORES)

    x_d = nc.dram_tensor("x", [CL, 128, HK, W], DT, kind="ExternalInput")
    ahn_d = nc.dram_tensor("ahn", [128, HK, NRP], DT, kind="ExternalInput")
    awn_d = nc.dram_tensor("awn", [128, WK, NCP], DT, kind="ExternalInput")
    awtb_d = nc.dram_tensor("awtb", [128, QBn, W], DT, kind="ExternalInput")
    ahtb_d = nc.dram_tensor("ahtb", [128, RBn, H], DT, kind="ExternalInput")
    o_d = nc.dram_tensor("o", [CL, 128, HK, W], DT, kind="ExternalOutput")

    with tile.TileContext(nc) as tc:
        with (
            tc.tile_pool(name="const", bufs=1) as constp,
            tc.tile_pool(name="xp", bufs=3) as xp,
            tc.tile_pool(name="rp", bufs=2) as rp,
            tc.tile_pool(name="sp", bufs=2) as sp,
            tc.tile_pool(name="up", bufs=2) as up,
            tc.tile_pool(name="op", bufs=2) as op_,
            # 8 PSUM banks total: psr 2 + pss 1 + psb 5 (stages 3+4 share)
            tc.tile_pool(name="psr", bufs=2, space="PSUM") as psr,
            tc.tile_pool(name="pss", bufs=1, space="PSUM") as pss,
            tc.tile_pool(name="psb", bufs=5, space="PSUM") as psb,
        ):
            # first channel's input in small pieces before the big broadcast
            # constants so the PE can start as early as possible (ahn is all
            # stage 1 needs); pieces land on different DMA queues and overlap
            xc0 = xp.tile([128, HK, W], DT)
            nc.sync.dma_start(xc0[:, 0:1, :], x_d[0][:, 0:1, :])
            ahn = constp.tile([128, HK, NRP], DT)
            nc.sync.dma_start(ahn[:], ahn_d[:])
            for k in range(1, HK):
                nc.sync.dma_start(xc0[:, k:k + 1, :], x_d[0][:, k:k + 1, :])
            awn = constp.tile([128, WK, NCP], DT)
            awtb = constp.tile([128, QBn, W], DT)
            ahtb = constp.tile([128, RBn, H], DT)

            def load_consts():
                # issued from the Act engine mid-s1(0): these transfers only
                # start once channel 0's input pieces are done with the bus
                nc.scalar.dma_start(awn[:], awn_d[:])
                nc.scalar.dma_start(awtb[:], awtb_d[:])
                nc.scalar.dma_start(ahtb[:], ahtb_d[:])

            # PSUM->SBUF casts alternate between Vector and Act so neither
            # bottlenecks (GpSimd cannot read PSUM on TRN2)
            def copier(eng):
                if eng % 2 == 0:
                    return nc.vector.tensor_copy
                return nc.scalar.copy

            # two stage-1 results fit one 2KB PSUM bank when NRP <= 256
            pair1 = 2 * NRP * 4 <= 2048

            xcs = [xc0] + [None] * (CL - 1)
            rcs = [None] * CL
            scs = [None] * CL
            ucs = [None] * CL

            def s1(c):
                """R^T[w, r] per W-chunk m (contract H in 6 chunks)."""
                xc = xcs[c]
                rc = rp.tile([128, WK, NRP], DT, tag="rc", name="rc")
                rcs[c] = rc
                if pair1:
                    for mp in range(WK // 2):
                        pr = psr.tile([128, 2, NRP], F32, tag="pr", name="pr")
                        for half in range(2):
                            m = 2 * mp + half
                            for k in range(HK):
                                nc.tensor.matmul(
                                    pr[:, half, :],
                                    xc[:, k, 128 * m:128 * m + 128],
                                    ahn[:, k, :],
                                    start=(k == 0), stop=(k == HK - 1),
                                )
                        copier(mp)(rc[:, 2 * mp:2 * mp + 2, :], pr[:])
                        if c == 0 and mp == 1:
                            load_consts()
                else:
                    for m in range(WK):
                        pr = psr.tile([128, NRP], F32, tag="pr", name="pr")
                        for k in range(HK):
                            nc.tensor.matmul(
                                pr[:],
                                xc[:, k, 128 * m:128 * m + 128],
                                ahn[:, k, :],
                                start=(k == 0), stop=(k == HK - 1),
                            )
                        copier(m)(rc[:, m, :], pr[:])

            def s2(c):
                """S^T[q, r] per q-block (contract W in 6 chunks); 1/n_q is
                baked into awn so the copy-out is a plain cast."""
                rc = rcs[c]
                sc = sp.tile([128, QBn, NRP], DT, tag="sc", name="sc")
                scs[c] = sc
                for b, (qo, qs) in enumerate(qblocks):
                    ps = pss.tile([128, NRP], F32, tag="ps", name="ps")
                    for k in range(WK):
                        nc.tensor.matmul(
                            ps[0:qs, :],
                            awn[:, k, qo:qo + qs],
                            rc[:, k, :],
                            start=(k == 0), stop=(k == WK - 1),
                        )
                    copier(b)(sc[0:qs, b, :], ps[0:qs, :])

            def s3(c):
                """U[r, j] = S^T[col_ids(j), r] per adapted r-block."""
                sc = scs[c]
                uc = up.tile([128, RBn, W], DT, tag="uc", name="uc")
                ucs[c] = uc
                for ri, (ro, rs) in enumerate(rblocks):
                    for n in range(W // NB):
                        pu = psb.tile([128, NB], F32, tag="pb", name="pu")
                        ks = qassign[n]
                        for j, b in enumerate(ks):
                            qo, qs = qblocks[b]
                            nc.tensor.matmul(
                                pu[0:rs, :],
                                sc[0:qs, b, ro:ro + rs],
                                awtb[0:qs, b, NB * n:NB * n + NB],
                                start=(j == 0), stop=(j == len(ks) - 1),
                            )
                        copier(2 * ri + n)(uc[0:rs, ri, NB * n:NB * n + NB],
                                           pu[0:rs, :])

            def s4(c):
                """OUT[i, j] via row gather; output DMA'd in 2-chunk pieces."""
                uc = ucs[c]
                ocC = op_.tile([128, HK, W], DT, tag="oc", name="ocC")
                for m in range(HK):
                    ks = rassign[m]
                    for n in range(W // NB):
                        po = psb.tile([128, NB], F32, tag="pb", name="po")
                        for j, b in enumerate(ks):
                            ro, rs = rblocks[b]
                            nc.tensor.matmul(
                                po[:],
                                ahtb[0:rs, b, 128 * m:128 * m + 128],
                                uc[0:rs, b, NB * n:NB * n + NB],
                                start=(j == 0), stop=(j == len(ks) - 1),
                            )
                        copier(2 * m + n)(ocC[:, m, NB * n:NB * n + NB],
                                          po[:])
                    if c == CL - 1:
                        nc.sync.dma_start(o_d[c][:, m:m + 1, :],
                                          ocC[:, m:m + 1, :])
                    elif m % 2 == 1:
                        nc.sync.dma_start(o_d[c][:, m - 1:m + 1, :],
                                          ocC[:, m - 1:m + 1, :])

            # software-pipelined channel loop: the PE consumes each stage's
            # PSUM->SBUF casts ~3us after they were issued, so it never stalls
            for c in range(CL + 1):
                if c < CL:
                    s1(c)
                    if c + 1 < CL:
                        xn = xp.tile([128, HK, W], DT, tag="xc", name="xn")
                        for kk in range(0, HK, 2):
                            nc.sync.dma_start(xn[:, kk:kk + 2, :],
                                              x_d[c + 1][:, kk:kk + 2, :])
                        xcs[c + 1] = xn
                if c >= 1:
                    s3(c - 1)
                    s4(c - 1)
                if c < CL:
                    s2(c)

    nc.compile()
    return nc


def _get_program(key):
    if key not in _cached:
        _cached[key] = _build_program(key)
    return _cached[key]


def _prepare(input, h_mask, v_mask):
    x = np.asarray(input, dtype=np.float32)
    hm = np.asarray(h_mask, dtype=np.int32)
    vm = np.asarray(v_mask, dtype=np.int32)
    assert x.shape == (1, H, W, C), x.shape

    row_ids = _segment_ids(hm[0])
    col_ids = _segment_ids(vm[0])
    nr = int(row_ids[-1]) + 1
    ncs = int(col_ids[-1]) + 1
    NRP = ((nr + 63) // 64) * 64
    NCP = ((ncs + 63) // 64) * 64

    rblocks, rassign = _adapt_blocks(row_ids, 128, nr)
    qblocks, qassign = _adapt_blocks(col_ids, NB, ncs)

    # Pad adapted blocks to 128 ids (overlapping is fine on the single-block
    # fast path: the gather matrices' extra rows are all-zero for the tiles
    # assigned to the block). Narrow-partition matmuls (M or K = 62) measure
    # ~1.5x slower per column on hardware than full 128-wide ones.
    def _pad_blocks(blocks, assign, npad):
        if any(len(a) != 1 for a in assign):
            return blocks   # multi-block accumulation needs disjoint blocks
        wmax = min(128, npad)
        return [(min(o, npad - wmax), wmax) for o, w in blocks]

    rblocks = _pad_blocks(rblocks, rassign, NRP)
    qblocks = _pad_blocks(qblocks, qassign, NCP)
    key = (NRP, NCP,
           tuple(rblocks), tuple(tuple(a) for a in rassign),
           tuple(qblocks), tuple(tuple(a) for a in qassign))

    n_r = np.bincount(row_ids, minlength=NRP).astype(np.float64)
    n_q = np.bincount(col_ids, minlength=NCP).astype(np.float64)

    # stage-1/2 weights: one-hot scaled by 1/n_r and 1/n_q (separable mean)
    ah = np.zeros((H, NRP), np.float32)
    ah[np.arange(H), row_ids] = (1.0 / n_r[row_ids])
    aw = np.zeros((W, NCP), np.float32)
    aw[np.arange(W), col_ids] = (1.0 / n_q[col_ids])
    # stage-3/4 gathers: exact one-hot
    ah1 = np.zeros((H, NRP), np.float32)
    ah1[np.arange(H), row_ids] = 1.0
    aw1 = np.zeros((W, NCP), np.float32)
    aw1[np.arange(W), col_ids] = 1.0

    # per-adapted-block partition layouts (zero padded to 128 partitions)
    QBn, RBn = len(qblocks), len(rblocks)
    awtb_dev = np.zeros((128, QBn, W), np.float32)
    for b, (qo, qs) in enumerate(qblocks):
        awtb_dev[0:qs, b, :] = aw1.T[qo:qo + qs]
    ahtb_dev = np.zeros((128, RBn, H), np.float32)
    for b, (ro, rs) in enumerate(rblocks):
        ahtb_dev[0:rs, b, :] = ah1.T[ro:ro + rs]

    ahn_dev = np.ascontiguousarray(
        ah.reshape(HK, 128, NRP).transpose(1, 0, 2)).astype(NPDT)
    awn_dev = np.ascontiguousarray(
        aw.reshape(WK, 128, NCP).transpose(1, 0, 2)).astype(NPDT)
    awtb_dev = awtb_dev.astype(NPDT)
    ahtb_dev = ahtb_dev.astype(NPDT)

    # per-core planar input: [CL, 128(p), HK(h0), W] with h = 128*h0 + p
    x64 = x[0].transpose(2, 0, 1)  # [C, H, W]
    in_maps = []
    for core in range(NCORES):
        xc = x64[CL * core:CL * (core + 1)]  # [CL, H, W]
        xdev = np.ascontiguousarray(
            xc.reshape(CL, HK, 128, W).transpose(0, 2, 1, 3)).astype(NPDT)
        in_maps.append({
            "x": xdev,
            "ahn": ahn_dev,
            "awn": awn_dev,
            "awtb": awtb_dev,
            "ahtb": ahtb_dev,
        })
    return in_maps, key


def _assemble(results):
    out = np.empty((1, H, W, C), np.float32)
    for core in range(NCORES):
        o = np.asarray(results[core]["o"]).astype(np.float32)  # [CL,128,HK,W]
        oc = o.transpose(0, 2, 1, 3).reshape(CL, H, W)         # h = 128*m + p
        out[0, :, :, CL * core:CL * (core + 1)] = oc.transpose(1, 2, 0)
    return out


def run(inputs: dict, trace: bool = False, **kwargs):
    """Full pipeline; returns (output, BassKernelResults)."""
    in_maps, key = _prepare(**inputs)
    nc = _get_program(key)
    res = run_bass_kernel_spmd(nc, in_maps, list(range(NCORES)),
                               trace=trace, **kwargs)
    return _assemble(res.results), res


def kernel(**inputs) -> np.ndarray:
    out, _ = run(inputs, trace=False)
    return out


# revision 12
# speedup vs baseline: 1.1234x; 1.0189x over previous
"""Grid pooling (segment mean over rectangular grid cells) on 8 trn2 cores.

Math: row/col masks induce contiguous run-segments along H and W, so every
grid cell is a rectangle and the whole op factorizes per channel as

    out_c = A_h @ ( A_h'^T @ X_c @ A_w' ) @ A_w^T

with segment-assignment matrices built on host from the tiny masks. The
mean scale 1/(n_r*n_q) is separable: 1/n_r is baked into the stage-1
weights A_h' and 1/n_q into the stage-2 weights A_w' (bf16), so every
PSUM->SBUF move is a plain cast. Channels (64) are sharded 8-way across
cores; each core runs 8 independent 768x768 channel planes through 4
matmul stages:

  1. R^T  = X_c^T  @ A_h'   (row-segment sums;   lhsT = X chunks)
  2. S^T  = A_w'^T @ R^T    (col-segment means;  lhsT = A_w' chunks)
  3. U    = S^T-gather cols (adapted r-blocks, single-k matmuls)
  4. OUT  = A_h-gather rows (per h-chunk)

Stage-1/2 matmuls stream ~192 columns, which exactly hides the ~75ns
LDWEIGHTS of each 128-row weight load and keeps the PE p-state at full
clock. The channel loop is software-pipelined as

    s1(c) -> s3(c-1) -> s4(c-1) -> s2(c)

so the PE never waits for a PSUM->SBUF cast to finish (each stage's
consumer runs ~3us after its producer). Casts alternate between the
Vector and Act engines (GpSimd cannot read PSUM on TRN2); stages 3+4
share one 5-deep PSUM pool so casts never backpressure the PE.
"""

import numpy as np
import ml_dtypes

from concourse import bacc, tile
import concourse.mybir as mybir
from concourse.bass_utils import run_bass_kernel_spmd

H = 768
W = 768
C = 64
NCORES = 8
CL = C // NCORES          # channels per core
HK = H // 128             # 6 H-chunks (contraction / output chunks)
WK = W // 128             # 6 W-chunks
NB = 384                  # free-dim tile for broadcast stages (768 = 2*384)

DT = mybir.dt.bfloat16
F32 = mybir.dt.float32
NPDT = ml_dtypes.bfloat16

_cached = {}


def _segment_ids(mask: np.ndarray) -> np.ndarray:
    """mask [L] binary -> segment ids via rising edges (pixel 0 -> seg 0)."""
    m = mask.astype(np.int64)
    prev = np.concatenate([[0], m[:-1]])
    rising = (m == 1) & (prev == 0)
    rising[0] = False
    return np.cumsum(rising.astype(np.int64)).astype(np.int32)


def _adapt_blocks(ids: np.ndarray, tile_len: int, nseg: int):
    """Partition the segment axis into blocks of <=128 ids such that the id
    range of every output tile of `tile_len` positions lies in one block.

    Returns (blocks, assign): blocks = [(start, width), ...] (may overlap by
    a shared boundary segment), assign[t] = [(block_idx), ...] the block(s)
    tile t accumulates over. Single-element lists on the fast path; falls
    back to fixed disjoint 128-blocks (multi-k) if a range is too wide.
    """
    L = len(ids)
    ntiles = L // tile_len
    ranges = [(int(ids[t * tile_len]), int(ids[(t + 1) * tile_len - 1]))
              for t in range(ntiles)]
    if all(hi - lo + 1 <= 128 for lo, hi in ranges):
        blocks, assign = [], []
        cur_start, cur_end = 0, -1
        for lo, hi in ranges:
            if hi - cur_start + 1 <= 128 and cur_end >= 0:
                cur_end = max(cur_end, hi)
            else:
                if cur_end >= 0:
                    blocks.append((cur_start, cur_end - cur_start + 1))
                cur_start, cur_end = lo, hi
            assign.append(len(blocks))
        blocks.append((cur_start, cur_end - cur_start + 1))
        return blocks, [[a] for a in assign]
    blocks = [(off, min(128, nseg - off)) for off in range(0, nseg, 128)]
    assign = []
    for lo, hi in ranges:
        assign.append([b for b, (s, w) in enumerate(blocks)
                       if not (hi < s or lo > s + w - 1)])
    return blocks, assign


def _build_program(key):
    (NRP, NCP, rblocks, rassign, qblocks, qassign) = key
    RBn = len(rblocks)
    QBn = len(qblocks)

    nc = bacc.Bacc("TRN2", target_bir_lowering=False, debug=False,
                   num_devices=NCORES)

    x_d = nc.dram_tensor("x", [CL, 128, HK, W], DT, kind="ExternalInput")
    ahn_d = nc.dram_tensor("ahn", [128, HK, NRP], DT, kind="ExternalInput")
    awn_d = nc.dram_tensor("awn", [128, WK, NCP], DT, kind="ExternalInput")
    awtb_d = nc.dram_tensor("awtb", [128, QBn, W], DT, kind="ExternalInput")
    ahtb_d = nc.dram_tensor("ahtb", [128, RBn, H], DT, kind="ExternalInput")
    o_d = nc.dram_tensor("o", [CL, 128, HK, W], DT, kind="ExternalOutput")

    with tile.TileContext(nc) as tc:
        with (
            tc.tile_pool(name="const", bufs=1) as constp,
            tc.tile_pool(name="xp", bufs=3) as xp,
            tc.tile_pool(name="rp", bufs=2) as rp,
            tc.tile_pool(name="sp", bufs=2) as sp,
            tc.tile_pool(name="up", bufs=2) as up,
            tc.tile_pool(name="op", bufs=2) as op_,
            # 8 PSUM banks total: psr 2 + pss 1 + psb 5 (stages 3+4 share)
            tc.tile_pool(name="psr", bufs=2, space="PSUM") as psr,
            tc.tile_pool(name="pss", bufs=1, space="PSUM") as pss,
            tc.tile_pool(name="psb", bufs=5, space="PSUM") as psb,
        ):
            # first channel's input in small pieces before the big broadcast
            # constants so the PE can start as early as possible (ahn is all
            # stage 1 needs); pieces land on different DMA queues and overlap
            xc0 = xp.tile([128, HK, W], DT)
            nc.sync.dma_start(xc0[:, 0:2, :], x_d[0][:, 0:2, :])
            ahn = constp.tile([128, HK, NRP], DT)
            nc.sync.dma_start(ahn[:], ahn_d[:])
            nc.sync.dma_start(xc0[:, 2:4, :], x_d[0][:, 2:4, :])
            nc.sync.dma_start(xc0[:, 4:6, :], x_d[0][:, 4:6, :])
            awn = constp.tile([128, WK, NCP], DT)
            awtb = constp.tile([128, QBn, W], DT)
            ahtb = constp.tile([128, RBn, H], DT)

            def load_consts():
                # issued from the Act engine mid-s1(0): these transfers only
                # start once channel 0's input pieces are done with the bus
                nc.scalar.dma_start(awn[:], awn_d[:])
                nc.scalar.dma_start(awtb[:], awtb_d[:])
                nc.scalar.dma_start(ahtb[:], ahtb_d[:])

            # PSUM->SBUF casts alternate between Vector and Act so neither
            # bottlenecks (GpSimd cannot read PSUM on TRN2)
            def copier(eng):
                if eng % 2 == 0:
                    return nc.vector.tensor_copy
                return nc.scalar.copy

            # two stage-1 results fit one 2KB PSUM bank when NRP <= 256
            pair1 = 2 * NRP * 4 <= 2048

            xcs = [xc0] + [None] * (CL - 1)
            rcs = [None] * CL
            scs = [None] * CL
            ucs = [None] * CL

            def s1(c):
                """R^T[w, r] per W-chunk m (contract H in 6 chunks)."""
                xc = xcs[c]
                rc = rp.tile([128, WK, NRP], DT, tag="rc", name="rc")
                rcs[c] = rc
                if pair1:
                    for mp in range(WK // 2):
                        pr = psr.tile([128, 2, NRP], F32, tag="pr", name="pr")
                        for half in range(2):
                            m = 2 * mp + half
                            for k in range(HK):
                                nc.tensor.matmul(
                                    pr[:, half, :],
                                    xc[:, k, 128 * m:128 * m + 128],
                                    ahn[:, k, :],
                                    start=(k == 0), stop=(k == HK - 1),
                                )
                        copier(mp)(rc[:, 2 * mp:2 * mp + 2, :], pr[:])
                        if c == 0 and mp == 1:
                            load_consts()
                else:
                    for m in range(WK):
                        pr = psr.tile([128, NRP], F32, tag="pr", name="pr")
                        for k in range(HK):
                            nc.tensor.matmul(
                                pr[:],
                                xc[:, k, 128 * m:128 * m + 128],
                                ahn[:, k, :],
                                start=(k == 0), stop=(k == HK - 1),
                            )
                        copier(m)(rc[:, m, :], pr[:])

            def s2(c):
                """S^T[q, r] per q-block (contract W in 6 chunks); 1/n_q is
                baked into awn so the copy-out is a plain cast."""
                rc = rcs[c]
                sc = sp.tile([128, QBn, NRP], DT, tag="sc", name="sc")
                scs[c] = sc
                for b, (qo, qs) in enumerate(qblocks):
                    ps = pss.tile([128, NRP], F32, tag="ps", name="ps")
                    for k in range(WK):
                        nc.tensor.matmul(
                            ps[0:qs, :],
                            awn[:, k, qo:qo + qs],
                            rc[:, k, :],
                            start=(k == 0), stop=(k == WK - 1),
                        )
                    copier(b)(sc[0:qs, b, :], ps[0:qs, :])

            def s3(c):
                """U[r, j] = S^T[col_ids(j), r] per adapted r-block."""
                sc = scs[c]
                uc = up.tile([128, RBn, W], DT, tag="uc", name="uc")
                ucs[c] = uc
                for ri, (ro, rs) in enumerate(rblocks):
                    for n in range(W // NB):
                        pu = psb.tile([128, NB], F32, tag="pb", name="pu")
                        ks = qassign[n]
                        for j, b in enumerate(ks):
                            qo, qs = qblocks[b]
                            nc.tensor.matmul(
                                pu[0:rs, :],
                                sc[0:qs, b, ro:ro + rs],
                                awtb[0:qs, b, NB * n:NB * n + NB],
                                start=(j == 0), stop=(j == len(ks) - 1),
                            )
                        copier(2 * ri + n)(uc[0:rs, ri, NB * n:NB * n + NB],
                                           pu[0:rs, :])

            def s4(c):
                """OUT[i, j] via row gather; output DMA'd in 2-chunk pieces."""
                uc = ucs[c]
                ocC = op_.tile([128, HK, W], DT, tag="oc", name="ocC")
                for m in range(HK):
                    ks = rassign[m]
                    for n in range(W // NB):
                        po = psb.tile([128, NB], F32, tag="pb", name="po")
                        for j, b in enumerate(ks):
                            ro, rs = rblocks[b]
                            nc.tensor.matmul(
                                po[:],
                                ahtb[0:rs, b, 128 * m:128 * m + 128],
                                uc[0:rs, b, NB * n:NB * n + NB],
                                start=(j == 0), stop=(j == len(ks) - 1),
                            )
                        copier(2 * m + n)(ocC[:, m, NB * n:NB * n + NB],
                                          po[:])
                    if m % 2 == 1:
                        nc.sync.dma_start(o_d[c][:, m - 1:m + 1, :],
                                          ocC[:, m - 1:m + 1, :])

            # software-pipelined channel loop: the PE consumes each stage's
            # PSUM->SBUF casts ~3us after they were issued, so it never stalls
            for c in range(CL + 1):
                if c < CL:
                    s1(c)
                    if c + 1 < CL:
                        xn = xp.tile([128, HK, W], DT, tag="xc", name="xn")
                        nc.sync.dma_start(xn[:], x_d[c + 1])
                        xcs[c + 1] = xn
                if c >= 1:
                    s3(c - 1)
                    s4(c - 1)
                if c < CL:
                    s2(c)

    nc.compile()
    return nc


def _get_program(key):
    if key not in _cached:
        _cached[key] = _build_program(key)
    return _cached[key]


def _prepare(input, h_mask, v_mask):
    x = np.asarray(input, dtype=np.float32)
    hm = np.asarray(h_mask, dtype=np.int32)
    vm = np.asarray(v_mask, dtype=np.int32)
    assert x.shape == (1, H, W, C), x.shape

    row_ids = _segment_ids(hm[0])
    col_ids = _segment_ids(vm[0])
    nr = int(row_ids[-1]) + 1
    ncs = int(col_ids[-1]) + 1
    NRP = ((nr + 63) // 64) * 64
    NCP = ((ncs + 63) // 64) * 64

    rblocks, rassign = _adapt_blocks(row_ids, 128, nr)
    qblocks, qassign = _adapt_blocks(col_ids, NB, ncs)

    # Pad adapted blocks to 128 ids (overlapping is fine on the single-block
    # fast path: the gather matrices' extra rows are all-zero for the tiles
    # assigned to the block). Narrow-partition matmuls (M or K = 62) measure
    # ~1.5x slower per column on hardware than full 128-wide ones.
    def _pad_blocks(blocks, assign, npad):
        if any(len(a) != 1 for a in assign):
            return blocks   # multi-block accumulation needs disjoint blocks
        wmax = min(128, npad)
        return [(min(o, npad - wmax), wmax) for o, w in blocks]

    rblocks = _pad_blocks(rblocks, rassign, NRP)
    qblocks = _pad_blocks(qblocks, qassign, NCP)
    key = (NRP, NCP,
           tuple(rblocks), tuple(tuple(a) for a in rassign),
           tuple(qblocks), tuple(tuple(a) for a in qassign))

    n_r = np.bincount(row_ids, minlength=NRP).astype(np.float64)
    n_q = np.bincount(col_ids, minlength=NCP).astype(np.float64)

    # stage-1/2 weights: one-hot scaled by 1/n_r and 1/n_q (separable mean)
    ah = np.zeros((H, NRP), np.float32)
    ah[np.arange(H), row_ids] = (1.0 / n_r[row_ids])
    aw = np.zeros((W, NCP), np.float32)
    aw[np.arange(W), col_ids] = (1.0 / n_q[col_ids])
    # stage-3/4 gathers: exact one-hot
    ah1 = np.zeros((H, NRP), np.float32)
    ah1[np.arange(H), row_ids] = 1.0
    aw1 = np.zeros((W, NCP), np.float32)
    aw1[np.arange(W), col_ids] = 1.0

    # per-adapted-block partition layouts (zero padded to 128 partitions)
    QBn, RBn = len(qblocks), len(rblocks)
    awtb_dev = np.zeros((128, QBn, W), np.float32)
    for b, (qo, qs) in enumerate(qblocks):
        awtb_dev[0:qs, b, :] = aw1.T[qo:qo + qs]
    ahtb_dev = np.zeros((128, RBn, H), np.float32)
    for b, (ro, rs) in enumerate(rblocks):
        ahtb_dev[0:rs, b, :] = ah1.T[ro:ro + rs]

    ahn_dev = np.ascontiguousarray(
        ah.reshape(HK, 128, NRP).transpose(1, 0, 2)).astype(NPDT)
    awn_dev = np.ascontiguousarray(
        aw.reshape(WK, 128, NCP).transpose(1, 0, 2)).astype(NPDT)
    awtb_dev = awtb_dev.astype(NPDT)
    ahtb_dev = ahtb_dev.astype(NPDT)

    # per-core planar input: [CL, 128(p), HK(h0), W] with h = 128*h0 + p
    x64 = x[0].transpose(2, 0, 1)  # [C, H, W]
    in_maps = []
    for core in range(NCORES):
        xc = x64[CL * core:CL * (core + 1)]  # [CL, H, W]
        xdev = np.ascontiguousarray(
            xc.reshape(CL, HK, 128, W).transpose(0, 2, 1, 3)).astype(NPDT)
        in_maps.append({
            "x": xdev,
            "ahn": ahn_dev,
            "awn": awn_dev,
            "awtb": awtb_dev,
            "ahtb": ahtb_dev,
        })
    return in_maps, key


def _assemble(results):
    out = np.empty((1, H, W, C), np.float32)
    for core in range(NCORES):
        o = np.asarray(results[core]["o"]).astype(np.float32)  # [CL,128,HK,W]
        oc = o.transpose(0, 2, 1, 3).reshape(CL, H, W)         # h = 128*m + p
        out[0, :, :, CL * core:CL * (core + 1)] = oc.transpose(1, 2, 0)
    return out


def run(inputs: dict, trace: bool = False, **kwargs):
    """Full pipeline; returns (output, BassKernelResults)."""
    in_maps, key = _prepare(**inputs)
    nc = _get_program(key)
    res = run_bass_kernel_spmd(nc, in_maps, list(range(NCORES)),
                               trace=trace, **kwargs)
    return _assemble(res.results), res


def kernel(**inputs) -> np.ndarray:
    out, _ = run(inputs, trace=False)
    return out
